# revision 1
# baseline (speedup 1.0000x reference)
"""Trainium2 Bass kernel for CombinedAttnProcessor (single-stream diffusion attn
with LoRA + RMSNorm + RoPE + IP-adapter branch).

Sharding: tensor-parallel over heads. 24 heads / 8 cores = 3 heads per core.
Each core computes q/k/v projections (+LoRA on the cond rows), per-head
RMSNorm+RoPE, block-masked attention and the IP-adapter attention for its 3
heads, producing a [3072, 384] slice of the output. Host concatenates slices.

All matmuls run in bf16 (fp32 PSUM accumulation). The attention mask is
handled structurally: rows < 2048 attend all 3072 keys, rows >= 2048 attend
only keys 2048:3072 — no -1e20 additions. Softmax skips max-subtraction
(post-RMSNorm scores are bounded, exp <= ~1e10, safe in fp32/bf16). Row sums
come free via a ones-column appended to V.

PSUM (8 banks), two sequential pools with manually juggled one-buffer tags:
  phase1 (ps1): projq t0/t1 (j parity), projk t2/t3, projv t4/t5,
                lora-down + transposes t6/t7.
  phase2 (ps2): score pairs sc0/sc1 ([128,1024] = 2 banks each, exp'd in one
                ACTIVATE to amortize the per-instruction pipe-fill),
                attn+ip accumulators c0..c3 (1 bank each; ones-column rowsums).
"""

import numpy as np
import ml_dtypes

import concourse.bass as bass
import concourse.tile as tile
from concourse import bacc, mybir
from concourse.bass_utils import run_bass_kernel_spmd
from concourse.masks import make_identity

F32 = mybir.dt.float32
BF16 = mybir.dt.bfloat16
AF = mybir.ActivationFunctionType
ALU = mybir.AluOpType

S = 3072
D = 3072
NH = 3            # heads per core
HD = 128
DC = NH * HD      # 384 output cols per core
NCH = D // 128    # 24 contraction chunks
BLOCK = 2048
COND = 1024
R = 64            # lora rank
TIP = 128         # ip tokens
SM = 1.0 / float(np.sqrt(HD))
EPS_QK = 1e-6
EPS_IP = 1e-5
N_CORES = 8

_BUILT = {}


def _bcast_rows(ap, parts=128):
    """Broadcast a 1-D DRAM tensor across `parts` partitions."""
    return bass.AP(tensor=ap.tensor, offset=ap.offset, ap=[[0, parts], *ap.ap])


def _build():
    nc = bacc.Bacc("TRN2", target_bir_lowering=False, debug=False,
                   num_devices=N_CORES)

    din = {}
    for name, shape, dt in [
        ("hsT", [D, S], BF16),
        ("wqT", [D, DC], BF16), ("wkT", [D, DC], BF16), ("wvT", [D, DC], BF16),
        ("bq", [DC], F32), ("bk", [DC], F32), ("bv", [DC], F32),
        ("ldq", [D, R], BF16), ("ldk", [D, R], BF16), ("ldv", [D, R], BF16),
        ("luq", [R, DC], BF16), ("luk", [R, DC], BF16), ("luv", [R, DC], BF16),
        ("iembT", [D, TIP], BF16),
        ("wkipT", [D, DC], BF16), ("wvipT", [D, DC], BF16),
        ("cos", [S, HD], BF16), ("sinf", [S, HD], BF16),
    ]:
        din[name] = nc.dram_tensor(name, shape, dt, kind="ExternalInput").ap()
    out_d = nc.dram_tensor("out", [S, DC], F32, kind="ExternalOutput").ap()

    with tile.TileContext(nc) as tc:
        _body(nc, tc, din, out_d)
    nc.compile()
    return nc


def _body(nc, tc, din, out_d):
    P = ["q", "k", "v"]
    with (
        tc.tile_pool(name="const", bufs=1) as const,
        tc.tile_pool(name="persist", bufs=1) as persist,
        tc.tile_pool(name="wpool", bufs=1) as wpool,
        tc.tile_pool(name="hsblk", bufs=7) as hspool,
        tc.tile_pool(name="cspool", bufs=2) as cspool,
        tc.tile_pool(name="tmp", bufs=2) as tmp,
        tc.tile_pool(name="small", bufs=32) as small,
        tc.tile_pool(name="pt2", bufs=4) as pt2,
        tc.tile_pool(name="osm", bufs=4) as osm,
        tc.tile_pool(name="op2", bufs=4) as op2,
    ):
        # ---- constants ----
        ident = const.tile([128, 128], BF16, name="ident")
        make_identity(nc, ident[:, :])
        eps_qk = const.tile([128, 1], F32, name="eps_qk")
        nc.vector.memset(eps_qk[:, :], EPS_QK)
        eps_ip = const.tile([128, 1], F32, name="eps_ip")
        nc.vector.memset(eps_ip[:, :], EPS_IP)
        bias = {p: const.tile([128, DC], F32, name=f"b{p}") for p in P}
        for p, nm in zip(P, ["bq", "bk", "bv"]):
            nc.sync.dma_start(out=bias[p][:, :], in_=_bcast_rows(din[nm]))

        # persistent per-head attention operands
        qT = [persist.tile([128, S], BF16, name=f"qT{h}") for h in range(NH)]
        kT = [persist.tile([128, S], BF16, name=f"kT{h}") for h in range(NH)]
        v_sb = [persist.tile([128, NH, HD + 1], BF16, name=f"v{i}")
                for i in range(S // 128)]
        kipT = [persist.tile([128, TIP], BF16, name=f"kipT{h}") for h in range(NH)]
        vip = persist.tile([128, NH, HD + 1], BF16, name="vip")

        ps1_ctx = tc.tile_pool(name="ps1", bufs=1, space="PSUM")
        ps1 = ps1_ctx.__enter__()

        def pstile(tag, shape, dtype=F32):
            return ps1.tile(shape, dtype, tag=tag, name=tag)

        # startup: iemb first, then the IP streams (small, unblock PE fast);
        # the bulk w/hs loads come after, interleaved per chunk so block-0
        # projection matmuls start as soon as chunk 0 lands.
        iemb = wpool.tile([128, NCH, TIP], BF16, name="iemb")
        for g in range(4):
            nc.sync.dma_start(
                out=iemb[:, g * 6:(g + 1) * 6, :],
                in_=din["iembT"].rearrange("(c p) t -> p c t", p=128)
                [:, g * 6:(g + 1) * 6, :])

        # ---- IP-adapter projections + bulk-load interleave ----
        # Per chunk-group g: stream wipK-g, wipV-g (feeding the PE right away)
        # and the main w / hs-block-0 groups behind them on the queues.
        w_sb = {p: wpool.tile([128, NCH, DC], BF16, name=f"w{p}") for p in P}
        wsrc = dict(zip(P, ["wqT", "wkT", "wvT"]))
        hs_pre_tiles = []
        ps_ip = {nm: pstile(f"t{pi}", [128, DC])
                 for pi, nm in enumerate(["wkipT", "wvipT"])}
        wip_tiles = {}
        for g in range(4):
            for nm in ("wkipT", "wvipT"):
                wt = hspool.tile([128, 6, DC], BF16, tag="hs", name="wip")
                nc.sync.dma_start(
                    out=wt[:, :, :],
                    in_=din[nm].rearrange("(c p) o -> p c o", p=128)
                    [:, g * 6:(g + 1) * 6, :])
                wip_tiles[(nm, g)] = wt
            for p in P:
                nc.sync.dma_start(
                    out=w_sb[p][:, g * 6:(g + 1) * 6, :],
                    in_=din[wsrc[p]].rearrange("(c p) o -> p c o", p=128)
                    [:, g * 6:(g + 1) * 6, :])
            t = hspool.tile([128, 6, 512], BF16, tag="hs", name="hs")
            nc.sync.dma_start(out=t[:, :, :], in_=bass.AP(
                tensor=din["hsT"].tensor, offset=(g * 6 * 128) * S,
                ap=[[S, 128], [128 * S, 6], [1, 512]]))
            hs_pre_tiles.append(t)
            for nm in ("wkipT", "wvipT"):
                wt = wip_tiles[(nm, g)]
                for ci in range(6):
                    c = g * 6 + ci
                    nc.tensor.matmul(ps_ip[nm][:, :], lhsT=iemb[:, c, :],
                                     rhs=wt[:, ci, :],
                                     start=(c == 0), stop=(c == NCH - 1))
        hs_pre = lambda c: hs_pre_tiles[c // 6][:, c % 6, :]

        for nm, is_kip in [("wkipT", True), ("wvipT", False)]:
            ps = ps_ip[nm]
            if is_kip:
                sq = tmp.tile([128, DC], F32, tag="sq", name="sq")
                nc.scalar.activation(out=sq[:, :], in_=ps[:, :], func=AF.Square)
                msq = small.tile([128, NH], F32, tag="msq", name="msq")
                nc.vector.tensor_reduce(
                    out=msq[:, :],
                    in_=sq[:, :].rearrange("p (h e) -> p h e", h=NH),
                    axis=mybir.AxisListType.X, op=ALU.add)
                nc.scalar.activation(out=msq[:, :], in_=msq[:, :], func=AF.Sqrt,
                                     scale=1.0 / HD, bias=eps_ip[:, :])
                rstd = small.tile([128, NH], F32, tag="rstd", name="rstd")
                nc.vector.reciprocal(rstd[:, :], msq[:, :])
                kn = tmp.tile([128, DC], BF16, tag="kn", name="kn")
                for h in range(NH):
                    nc.vector.tensor_scalar_mul(kn[:, h * HD:(h + 1) * HD],
                                                ps[:, h * HD:(h + 1) * HD],
                                                rstd[:, h:h + 1])
                for h in range(NH):
                    pt = pstile(f"t{6 + h % 2}", [128, 128], BF16)
                    nc.tensor.transpose(pt[:, :], kn[:, h * HD:(h + 1) * HD],
                                        ident[:, :])
                    nc.vector.tensor_copy(kipT[h][:, :], pt[:, :])
            else:
                for h in range(NH):
                    nc.vector.tensor_copy(vip[:, h, 0:HD],
                                          ps[:, h * HD:(h + 1) * HD])
                    nc.gpsimd.memset(vip[:, h, HD:HD + 1], 1.0)

        # ---- remaining bulk loads ----
        def load_hs_block(s0):
            tiles = []
            for g in range(4):
                t = hspool.tile([128, 6, 512], BF16, tag="hs", name="hs")
                nc.sync.dma_start(out=t[:, :, :], in_=bass.AP(
                    tensor=din["hsT"].tensor,
                    offset=(g * 6 * 128) * S + s0,
                    ap=[[S, 128], [128 * S, 6], [1, 512]]))
                tiles.append(t)
            return lambda c: tiles[c // 6][:, c % 6, :]
        ldqk = wpool.tile([128, NCH, 2 * R], BF16, name="ldqk")
        nc.sync.dma_start(out=ldqk[:, :, 0:R],
                          in_=din["ldq"].rearrange("(c p) r -> p c r", p=128))
        nc.sync.dma_start(out=ldqk[:, :, R:2 * R],
                          in_=din["ldk"].rearrange("(c p) r -> p c r", p=128))
        ldv = wpool.tile([128, NCH, R], BF16, name="ldv")
        nc.sync.dma_start(out=ldv[:, :, :],
                          in_=din["ldv"].rearrange("(c p) r -> p c r", p=128))
        luqk = wpool.tile([128, DC], BF16, name="luqk")
        nc.sync.dma_start(out=luqk[0:R, :], in_=din["luq"])
        nc.sync.dma_start(out=luqk[R:2 * R, :], in_=din["luk"])
        luv = wpool.tile([R, DC], BF16, name="luv")
        nc.sync.dma_start(out=luv[:, :], in_=din["luv"])
        lu_rhs = {"q": lambda: luqk[0:R, :], "k": lambda: luqk[R:2 * R, :],
                  "v": lambda: luv[:, :]}

        # ---- q/k/v projections + LoRA + rmsnorm + rope + transposes ----
        ptag = {"q": ("t0", "t1"), "k": ("t2", "t3"), "v": ("t4", "t5")}
        for b in range(S // 512):
            s0 = b * 512
            hs_t = hs_pre if b == 0 else load_hs_block(s0)
            cond = s0 >= BLOCK
            if cond:
                # rows 0:64 = down_q ranks, 64:128 = down_k ranks
                pd = pstile("t6", [128, 512])
                for c in range(NCH):
                    nc.tensor.matmul(pd[:, :], lhsT=ldqk[:, c, :],
                                     rhs=hs_t(c),
                                     start=(c == 0), stop=(c == NCH - 1))
                dnqk = small.tile([128, 512], BF16, tag="dnqk", name="dnqk",
                                  bufs=1)
                nc.vector.tensor_copy(dnqk[:, :], pd[:, :])
                pdv = pstile("t7", [R, 512])
                for c in range(NCH):
                    nc.tensor.matmul(pdv[:, :], lhsT=ldv[:, c, :],
                                     rhs=hs_t(c),
                                     start=(c == 0), stop=(c == NCH - 1))
                dnv = small.tile([R, 512], BF16, tag="dnv", name="dnv", bufs=1)
                nc.vector.tensor_copy(dnv[:, :], pdv[:, :])
                dn_lhsT = {"q": lambda jsl: dnqk[0:R, jsl],
                           "k": lambda jsl: dnqk[R:2 * R, jsl],
                           "v": lambda jsl: dnv[:, jsl]}

            cos_b = cspool.tile([128, 4, HD], BF16, tag="cos", name="cos")
            nc.sync.dma_start(out=cos_b[:, :, :], in_=bass.AP(
                tensor=din["cos"].tensor, offset=b * 512 * HD,
                ap=[[HD, 128], [128 * HD, 4], [1, HD]]))
            sin_b = cspool.tile([128, 4, HD], BF16, tag="sin", name="sin")
            nc.sync.dma_start(out=sin_b[:, :, :], in_=bass.AP(
                tensor=din["sinf"].tensor, offset=b * 512 * HD,
                ap=[[HD, 128], [128 * HD, 4], [1, HD]]))
            for j in range(4):
                i = b * 4 + j
                js = slice(j * 128, (j + 1) * 128)
                cj = cos_b[:, j]
                sj = sin_b[:, j]
                # stride-0 head-broadcast views [p, NH, ...]
                cos3 = bass.AP(tensor=cj.tensor, offset=cj.offset,
                               ap=[cj.ap[0], [0, NH], [1, HD]])
                sin3p = bass.AP(tensor=sj.tensor, offset=sj.offset,
                                ap=[sj.ap[0], [0, NH], [2, HD // 2], [1, 2]])

                # c-outer / p-inner: one stationary hs tile feeds q,k,v
                psd = {p: pstile(ptag[p][j % 2], [128, DC]) for p in P}
                for c in range(NCH):
                    for p in P:
                        nc.tensor.matmul(psd[p][:, :], lhsT=hs_t(c)[:, js],
                                         rhs=w_sb[p][:, c, :],
                                         start=(c == 0),
                                         stop=(c == NCH - 1 and not cond))
                if cond:
                    for p in P:
                        nc.tensor.matmul(psd[p][:, :], lhsT=dn_lhsT[p](js),
                                         rhs=lu_rhs[p](),
                                         start=False, stop=True)
                for p in P:
                    ps = psd[p]
                    if p == "v":
                        for h in range(NH):
                            nc.vector.tensor_tensor(
                                out=v_sb[i][:, h, 0:HD],
                                in0=ps[:, h * HD:(h + 1) * HD],
                                in1=bias["v"][:, h * HD:(h + 1) * HD],
                                op=ALU.add)
                            nc.gpsimd.memset(v_sb[i][:, h, HD:HD + 1], 1.0)
                        continue
                    # q/k epilogue: bias, rmsnorm, rope, per-head transpose
                    qb = tmp.tile([128, DC], F32, tag="qb", name="qb")
                    nc.vector.tensor_tensor(out=qb[:, :], in0=ps[:, :],
                                            in1=bias[p][:, :], op=ALU.add)
                    sq = tmp.tile([128, DC], F32, tag="sq", name="sq")
                    nc.vector.tensor_mul(sq[:, :], qb[:, :], qb[:, :])
                    msq = small.tile([128, NH], F32, tag="msq", name="msq")
                    nc.vector.tensor_reduce(
                        out=msq[:, :],
                        in_=sq[:, :].rearrange("p (h e) -> p h e", h=NH),
                        axis=mybir.AxisListType.X, op=ALU.add)
                    nc.scalar.activation(out=msq[:, :], in_=msq[:, :],
                                         func=AF.Sqrt,
                                         scale=1.0 / HD, bias=eps_qk[:, :])
                    rstd = small.tile([128, NH], F32, tag="rstd", name="rstd")
                    nc.vector.reciprocal(rstd[:, :], msq[:, :])
                    qn = tmp.tile([128, DC], F32, tag="qn", name="qn")
                    for h in range(NH):
                        nc.vector.tensor_scalar_mul(qn[:, h * HD:(h + 1) * HD],
                                                    qb[:, h * HD:(h + 1) * HD],
                                                    rstd[:, h:h + 1])
                    # rope: m1 = qn*cos ; m2 = swap_pairs(qn)*sin_signfolded
                    m1 = tmp.tile([128, DC], F32, tag="sq", name="m1")
                    nc.vector.tensor_tensor(
                        out=m1[:, :].rearrange("p (h e) -> p h e", h=NH),
                        in0=qn[:, :].rearrange("p (h e) -> p h e", h=NH),
                        in1=cos3, op=ALU.mult)
                    m2 = tmp.tile([128, DC], F32, tag="qb", name="m2")
                    m2v = m2[:, :].rearrange("p (h i two) -> p h i two",
                                             h=NH, two=2)
                    qnv = qn[:, :].rearrange("p (h i two) -> p h i two",
                                             h=NH, two=2)
                    nc.vector.tensor_tensor(out=m2v[:, :, :, 0:1],
                                            in0=qnv[:, :, :, 1:2],
                                            in1=sin3p[:, :, :, 0:1],
                                            op=ALU.mult)
                    nc.vector.tensor_tensor(out=m2v[:, :, :, 1:2],
                                            in0=qnv[:, :, :, 0:1],
                                            in1=sin3p[:, :, :, 1:2],
                                            op=ALU.mult)
                    qr = tmp.tile([128, DC], BF16, tag="qr", name="qr")
                    nc.vector.tensor_tensor(out=qr[:, :], in0=m1[:, :],
                                            in1=m2[:, :], op=ALU.add)
                    dstT = qT if p == "q" else kT
                    for h in range(NH):
                        pt = pstile(f"t{6 + h % 2}", [128, 128], BF16)
                        nc.tensor.transpose(pt[:, :],
                                            qr[:, h * HD:(h + 1) * HD],
                                            ident[:, :])
                        nc.vector.tensor_copy(dstT[h][:, i * 128:(i + 1) * 128],
                                              pt[:, :])

        # ===== phase 2: attention =====
        # close phase-1 PSUM pool, open phase-2 layout:
        #   sc0/sc1: [128,1024] score pairs (2 banks each)
        #   c0..c3:  [128,258] attn+ip accumulators (1 bank each)
        ps1_ctx.__exit__(None, None, None)
        ps2_ctx = tc.tile_pool(name="ps2", bufs=1, space="PSUM")
        ps2 = ps2_ctx.__enter__()
        scn = 0
        for h in range(NH):
            for sbk in range(S // 512):
                s0 = sbk * 512
                tcs = (list(range(NCH)) if s0 < BLOCK
                       else list(range(BLOCK // 128, NCH)))
                pairs = [(tcs[x], tcs[x + 1]) for x in range(0, len(tcs), 2)]

                # main pairs, software-pipelined with 1 pair of score lookahead
                def sc_exp(pr):
                    nonlocal scn
                    tka, tkb = pr
                    psc = ps2.tile([128, 1024], F32, tag=f"sc{scn % 2}",
                                   name=f"sc{scn % 2}")
                    scn += 1
                    for half, tk in ((0, tka), (1, tkb)):
                        nc.tensor.matmul(psc[:, half * 512:(half + 1) * 512],
                                         lhsT=kT[h][:, tk * 128:(tk + 1) * 128],
                                         rhs=qT[h][:, s0:s0 + 512],
                                         start=True, stop=True,
                                         skip_group_check=True)
                    pT = pt2.tile([128, 1024], BF16, tag="pT", name="pT")
                    nc.scalar.activation(out=pT[:, :], in_=psc[:, :],
                                         func=AF.Exp, scale=SM)
                    return pT

                pT_q = [sc_exp(pairs[0])]
                cmb = [ps2.tile([128, 2 * (HD + 1)], F32, tag=f"c{j}",
                                name=f"c{j}") for j in range(4)]

                # ip branch: emitted after pair-0's score/exp (so ACT starts the
                # main exp stream immediately) but its attnV still precedes main
                # pair-0's start=True — ip writes cols 129:258 once; the later
                # main start clears only has_written bits, not data.
                psc = ps2.tile([128, 1024], F32, tag=f"sc{scn % 2}",
                               name=f"sc{scn % 2}")
                scn += 1
                nc.tensor.matmul(psc[:, 0:512], lhsT=kipT[h][:, :],
                                 rhs=qT[h][:, s0:s0 + 512],
                                 start=True, stop=True, skip_group_check=True)
                pTip = pt2.tile([128, 1024], BF16, tag="pT", name="pT")
                nc.scalar.activation(out=pTip[:, 0:512], in_=psc[:, 0:512],
                                     func=AF.Exp, scale=SM)
                for j in range(4):
                    nc.tensor.matmul(cmb[j][:, HD + 1:2 * (HD + 1)],
                                     lhsT=pTip[:, j * 128:(j + 1) * 128],
                                     rhs=vip[:, h, :],
                                     start=True, stop=True,
                                     skip_group_check=True)

                for pi2 in range(len(pairs)):
                    if pi2 + 1 < len(pairs):
                        pT_q.append(sc_exp(pairs[pi2 + 1]))
                    pT = pT_q.pop(0)
                    tka, tkb = pairs[pi2]
                    for half, tk in ((0, tka), (1, tkb)):
                        for j in range(4):
                            nc.tensor.matmul(
                                cmb[j][:, 0:HD + 1],
                                lhsT=pT[:, half * 512 + j * 128:
                                        half * 512 + (j + 1) * 128],
                                rhs=v_sb[tk][:, h, :],
                                start=(pi2 == 0 and half == 0),
                                stop=(pi2 == len(pairs) - 1 and half == 1),
                                skip_group_check=True)
                # normalize + combine + one batched store
                o2 = op2.tile([128, 4, HD], F32, tag="o2", name="o2", bufs=2)
                for j in range(4):
                    rm = osm.tile([128, 1], F32, tag="rm", name="rm")
                    nc.vector.reciprocal(rm[:, :], cmb[j][:, HD:HD + 1])
                    ri = osm.tile([128, 1], F32, tag="ri", name="ri")
                    nc.vector.reciprocal(ri[:, :],
                                         cmb[j][:, 2 * HD + 1:2 * HD + 2])
                    nc.vector.tensor_scalar_mul(o2[:, j, :], cmb[j][:, 0:HD],
                                                rm[:, :])
                    nc.vector.scalar_tensor_tensor(
                        out=o2[:, j, :], in0=cmb[j][:, HD + 1:2 * HD + 1],
                        scalar=ri[:, :], in1=o2[:, j, :],
                        op0=ALU.mult, op1=ALU.add)
                nc.sync.dma_start(
                    out=bass.AP(tensor=out_d.tensor,
                                offset=s0 * DC + h * HD,
                                ap=[[DC, 128], [128 * DC, 4], [1, HD]]),
                    in_=o2[:, :, :])
        ps2_ctx.__exit__(None, None, None)


def _prep_inputs(inputs):
    # Inputs may be jax arrays: np.asarray without an explicit dtype hits the
    # jax Array's cached host copy (an explicit dtype forces a fresh transfer).
    inp = {k: np.asarray(v) for k, v in inputs.items()}
    bf = lambda x: np.ascontiguousarray(x).astype(ml_dtypes.bfloat16)
    f32 = lambda x: np.ascontiguousarray(x, dtype=np.float32)
    hsT = bf(inp["hidden_states"][0].T)
    iembT = bf(inp["image_emb"][0].T)
    cos = bf(inp["rope_cos"])
    sin_sf = inp["rope_sin"].copy()
    sin_sf[:, 0::2] *= -1.0
    sinf = bf(sin_sf)
    # pre-transpose the full matrices once, then slice per core
    wT = {k: bf(inp[k].T) for k in ("wq", "wk", "wv", "w_kip", "w_vip")}
    luT = {k: bf(inp[k].T) for k in ("lq_up", "lk_up", "lv_up")}
    ldT = {k: bf(inp[k].T) for k in ("lq_down", "lk_down", "lv_down")}
    in_maps = []
    for c in range(N_CORES):
        sl = slice(DC * c, DC * (c + 1))
        m = {
            "hsT": hsT, "iembT": iembT, "cos": cos, "sinf": sinf,
            "wqT": np.ascontiguousarray(wT["wq"][:, sl]),
            "wkT": np.ascontiguousarray(wT["wk"][:, sl]),
            "wvT": np.ascontiguousarray(wT["wv"][:, sl]),
            "bq": f32(inp["bq"][sl]),
            "bk": f32(inp["bk"][sl]),
            "bv": f32(inp["bv"][sl]),
            "ldq": ldT["lq_down"], "ldk": ldT["lk_down"], "ldv": ldT["lv_down"],
            "luq": np.ascontiguousarray(luT["lq_up"][:, sl]),
            "luk": np.ascontiguousarray(luT["lk_up"][:, sl]),
            "luv": np.ascontiguousarray(luT["lv_up"][:, sl]),
            "wkipT": np.ascontiguousarray(wT["w_kip"][:, sl]),
            "wvipT": np.ascontiguousarray(wT["w_vip"][:, sl]),
        }
        in_maps.append(m)
    return in_maps


def kernel(**inputs):
    if "nc" not in _BUILT:
        _BUILT["nc"] = _build()
    nc = _BUILT["nc"]
    in_maps = _prep_inputs(inputs)
    res = run_bass_kernel_spmd(nc, in_maps, list(range(N_CORES))).results
    out = np.concatenate([res[c]["out"] for c in range(N_CORES)], axis=-1)
    return out.reshape(1, S, D).astype(np.float32)



# revision 32
# speedup vs baseline: 1.1074x; 1.1074x over previous
"""Trainium2 Bass kernel for CombinedAttnProcessor (single-stream diffusion attn
with LoRA + RMSNorm + RoPE + IP-adapter branch).

Sharding: tensor-parallel over heads. 24 heads / 8 cores = 3 heads per core.
Each core computes q/k/v projections (+LoRA on the cond rows), per-head
RMSNorm+RoPE, block-masked attention and the IP-adapter attention for its 3
heads, producing a [3072, 384] slice of the output. Host concatenates slices.

Projection-class matmuls (q/k/v, LoRA-down, IP) run in fp8e4 DoubleRow with
host-side residual splits: x ~ x1 + x2 (both fp8, x pre-scaled into fp8's
sweet range), w ~ w1 + w2, and the kernel accumulates x1w1 + x1w2 + x2w1
(three K=256 DoubleRow products per chunk pair instead of two bf16 K=128
matmuls), dropping only the ~(2^-9)^2 x2w2 term. PSUM carries the 2^10
operand prescale; epilogues fold the 2^-10 descale into their PSUM read.
Attention (scores / AV) stays bf16.

The attention mask is handled structurally: rows < 2048 attend all 3072 keys,
rows >= 2048 attend only keys 2048:3072 — no -1e20 additions. Softmax skips
max-subtraction (post-RMSNorm scores are bounded, exp <= ~1e10, safe in
fp32/bf16). Row sums come free via a ones-column appended to V.

PSUM (8 banks), two sequential pools with manually juggled one-buffer tags:
  phase1 (ps1): projq t0/t1 (j parity), projk t2/t3, projv t4/t5,
                lora-down + transposes t6/t7.
  phase2 (ps2): score pairs sc0/sc1 ([128,1024] = 2 banks each, exp'd in one
                ACTIVATE to amortize the per-instruction pipe-fill),
                attn+ip accumulators c0..c3 (1 bank each; ones-column rowsums).
"""

import numpy as np
import ml_dtypes

import concourse.bass as bass
import concourse.tile as tile
from concourse import bacc, mybir
from concourse.bass_utils import run_bass_kernel_spmd
from concourse.masks import make_identity

F32 = mybir.dt.float32
BF16 = mybir.dt.bfloat16
F8 = mybir.dt.float8e4
AF = mybir.ActivationFunctionType
ALU = mybir.AluOpType
DR = mybir.MatmulPerfMode.DoubleRow

S = 3072
D = 3072
NH = 3            # heads per core
HD = 128
DC = NH * HD      # 384 output cols per core
NCH = D // 128    # 24 contraction chunks
BLOCK = 2048
COND = 1024
R = 64            # lora rank
TIP = 128         # ip tokens
SM = 1.0 / float(np.sqrt(HD))
EPS_QK = 1e-6
EPS_IP = 1e-5
N_CORES = 8

# fp8 prescales (host) and the matching PSUM descale (device epilogues)
HS_S = 16.0       # hidden_states / image_emb prescale
W_S = 64.0        # weight prescale (wq/wk/wv, lora-down, w_kip/w_vip)
DESCALE = 1.0 / (HS_S * W_S)

_BUILT = {}


def _bcast_rows(ap, parts=128):
    """Broadcast a 1-D DRAM tensor across `parts` partitions."""
    return bass.AP(tensor=ap.tensor, offset=ap.offset, ap=[[0, parts], *ap.ap])


def _build():
    nc = bacc.Bacc("TRN2", target_bir_lowering=False, debug=False,
                   num_devices=N_CORES)

    din = {}
    specs = [
        ("bq", [DC], F32), ("bk", [DC], F32), ("bv", [DC], F32),
        ("luq", [R, DC], BF16), ("luk", [R, DC], BF16), ("luv", [R, DC], BF16),
        ("cos", [128, S], BF16), ("sinf", [128, S], BF16),
    ]
    # weight-class tensors arrive host-pre-tiled partition-major
    # ([128, NCH*cols]) so every DMA runs >=512B contiguous descriptors
    for base, shape in [
        ("hsT", [D, S]),
        ("wqT", [128, NCH * DC]), ("wkT", [128, NCH * DC]),
        ("wvT", [128, NCH * DC]),
        ("ldqk", [128, NCH * 2 * R]), ("ldv", [128, NCH * R]),
        ("iembT", [128, NCH * TIP]),
        ("wkipT", [128, NCH * DC]), ("wvipT", [128, NCH * DC]),
    ]:
        specs.append((base + "1", shape, F8))
        specs.append((base + "2", shape, F8))
    for name, shape, dt in specs:
        din[name] = nc.dram_tensor(name, shape, dt, kind="ExternalInput").ap()
    out_d = nc.dram_tensor("out", [S, DC], BF16, kind="ExternalOutput").ap()

    with tile.TileContext(nc) as tc:
        _body(nc, tc, din, out_d)
    nc.compile()
    return nc


def _body(nc, tc, din, out_d):
    P = ["q", "k", "v"]
    with (
        tc.tile_pool(name="const", bufs=1) as const,
        tc.tile_pool(name="persist", bufs=1) as persist,
        tc.tile_pool(name="wpool", bufs=1) as wpool,
        tc.tile_pool(name="hsblk", bufs=6) as hspool,
        tc.tile_pool(name="cspool", bufs=2) as cspool,
        tc.tile_pool(name="tmp", bufs=2) as tmp,
        tc.tile_pool(name="small", bufs=32) as small,
        tc.tile_pool(name="pt2", bufs=3) as pt2,
        tc.tile_pool(name="osm", bufs=4) as osm,
        tc.tile_pool(name="op2", bufs=4) as op2,
    ):
        # ---- constants ----
        ident = const.tile([128, 128], BF16, name="ident")
        make_identity(nc, ident[:, :])
        eps_qk = const.tile([128, 1], F32, name="eps_qk")
        nc.vector.memset(eps_qk[:, :], EPS_QK)
        eps_ip = const.tile([128, 1], F32, name="eps_ip")
        nc.vector.memset(eps_ip[:, :], EPS_IP)
        bias = {p: const.tile([128, DC], F32, name=f"b{p}") for p in P}
        for p, nm in zip(P, ["bq", "bk", "bv"]):
            nc.sync.dma_start(out=bias[p][:, :], in_=_bcast_rows(din[nm]))

        # persistent per-head attention operands
        qT = [persist.tile([128, S], BF16, name=f"qT{h}") for h in range(NH)]
        kT = [persist.tile([128, S], BF16, name=f"kT{h}") for h in range(NH)]
        v_sb = [persist.tile([128, NH, HD + 1], BF16, name=f"v{i}")
                for i in range(S // 128)]
        for t in v_sb:
            nc.gpsimd.memset(t[:, :, HD:HD + 1], 1.0)
        kipT = [persist.tile([128, TIP], BF16, name=f"kipT{h}") for h in range(NH)]
        vip = persist.tile([128, NH, HD + 1], BF16, name="vip")

        ps1_ctx = tc.tile_pool(name="ps1", bufs=1, space="PSUM")
        ps1 = ps1_ctx.__enter__()

        def pstile(tag, shape, dtype=F32):
            return ps1.tile(shape, dtype, tag=tag, name=tag)

        # startup: iemb first, then the IP streams (small, unblock PE fast);
        # the bulk w/hs loads come after, interleaved per chunk so block-0
        # projection matmuls start as soon as chunk 0 lands.
        iemb = {s: wpool.tile([128, NCH, TIP], F8, name=f"iemb{s}")
                for s in (1, 2)}
        for s in (1, 2):
            nc.sync.dma_start(out=iemb[s][:, :, :], in_=din[f"iembT{s}"])

        # hs blocks load as 4 half-tiles (2 planes x chunk-halves 0:12/12:24)
        # on their own ring; wip quarters ride a separate small ring so the
        # two never deadlock each other through shared slots.
        def load_hs_block(s0, emit=None):
            tiles = {1: [None, None], 2: [None, None]}

            def load(s, half):
                t = hspool.tile([128, 12, 512], F8, tag="hs", name="hs")
                nc.sync.dma_start(out=t[:, :, :], in_=bass.AP(
                    tensor=din[f"hsT{s}"].tensor,
                    offset=(half * 12 * 128) * S + s0,
                    ap=[[S, 128], [128 * S, 12], [1, 512]]))
                tiles[s][half] = t

            if emit is None:
                for half in (0, 1):
                    for s in (1, 2):
                        load(s, half)
            else:
                emit(load)
            return ({s: (lambda c, s=s: tiles[s][c // 12][:, c % 12, :])
                     for s in (1, 2)},
                    {s: (lambda c, s=s: tiles[s][c // 12]
                         [:, c % 12:c % 12 + 2, :]) for s in (1, 2)})

        # rope tables: host-pretiled [128, S]; per-block tiles loaded one
        # block ahead of use
        cs_tiles = {}

        def load_cs(b):
            cos_b = cspool.tile([128, 4, HD], BF16, tag="cos", name="cos")
            nc.sync.dma_start(out=cos_b[:, :, :],
                              in_=din["cos"][:, b * 512:(b + 1) * 512])
            sin_b = cspool.tile([128, 4, HD], BF16, tag="sin", name="sin")
            nc.sync.dma_start(out=sin_b[:, :, :],
                              in_=din["sinf"][:, b * 512:(b + 1) * 512])
            cs_tiles[b] = (cos_b, sin_b)

        # ---- IP-adapter projections + bulk-load interleave ----
        # Per chunk-group g: stream wipK-g, wipV-g (feeding the PE right away)
        # and the main w / hs-block-0 halves behind them on the queues.
        w_sb = {(p, s): wpool.tile([128, NCH, DC], F8, name=f"w{p}{s}")
                for p in P for s in (1, 2)}
        wsrc = dict(zip(P, ["wqT", "wkT", "wvT"]))
        ps_ip = {nm: pstile(f"t{pi}", [128, DC])
                 for pi, nm in enumerate(["wkipT", "wvipT"])}
        hs_b0 = {}

        def emit_b0(load):
            hs_b0["load"] = load
        hs_pre, hs_pre_pair = load_hs_block(0, emit=emit_b0)
        for g in range(4):
            gsl = slice(g * 6 * DC, (g + 1) * 6 * DC)
            wip_tiles = {}
            for nm in ("wkipT", "wvipT"):
                for s in (1, 2):
                    wt = hspool.tile([128, 6, DC], F8, tag="wip", name="wip",
                                     bufs=4)
                    nc.sync.dma_start(out=wt[:, :, :],
                                      in_=din[nm + str(s)][:, gsl])
                    wip_tiles[(nm, s)] = wt
            for p in P:
                for s in (1, 2):
                    nc.sync.dma_start(
                        out=w_sb[(p, s)][:, g * 6:(g + 1) * 6, :],
                        in_=din[wsrc[p] + str(s)][:, gsl])
            if g == 0:
                for s in (1, 2):
                    hs_b0["load"](s, 0)
                load_cs(0)
            elif g == 1:
                for s in (1, 2):
                    hs_b0["load"](s, 1)
                load_cs(1)
            # 3-term DoubleRow accumulation per chunk pair of this group
            for nm in ("wkipT", "wvipT"):
                w1 = wip_tiles[(nm, 1)]
                w2 = wip_tiles[(nm, 2)]
                for cp in range(3):
                    c = g * 6 + 2 * cp
                    last = (c == NCH - 2)
                    nc.tensor.matmul(ps_ip[nm][:, :],
                                     lhsT=iemb[1][:, c:c + 2, :],
                                     rhs=w1[:, 2 * cp:2 * cp + 2, :],
                                     start=(c == 0), stop=False, perf_mode=DR)
                    nc.tensor.matmul(ps_ip[nm][:, :],
                                     lhsT=iemb[1][:, c:c + 2, :],
                                     rhs=w2[:, 2 * cp:2 * cp + 2, :],
                                     start=False, stop=False, perf_mode=DR)
                    nc.tensor.matmul(ps_ip[nm][:, :],
                                     lhsT=iemb[2][:, c:c + 2, :],
                                     rhs=w1[:, 2 * cp:2 * cp + 2, :],
                                     start=False, stop=last, perf_mode=DR)

        for nm, is_kip in [("wkipT", True), ("wvipT", False)]:
            ps = ps_ip[nm]
            if is_kip:
                sq = tmp.tile([128, DC], F32, tag="sq", name="sq")
                nc.scalar.activation(out=sq[:, :], in_=ps[:, :], func=AF.Square)
                msq = small.tile([128, NH], F32, tag="msq", name="msq")
                nc.vector.tensor_reduce(
                    out=msq[:, :],
                    in_=sq[:, :].rearrange("p (h e) -> p h e", h=NH),
                    axis=mybir.AxisListType.X, op=ALU.add)
                # sq carries the 2^20 operand prescale; fold its removal into
                # the Sqrt's input scale so rms comes out unscaled.
                nc.scalar.activation(out=msq[:, :], in_=msq[:, :], func=AF.Sqrt,
                                     scale=DESCALE * DESCALE / HD,
                                     bias=eps_ip[:, :])
                rstd = small.tile([128, NH], F32, tag="rstd", name="rstd")
                nc.vector.reciprocal(rstd[:, :], msq[:, :])
                # kn = (2^10 x) * (rstd * 2^-10) -> normalized, unscaled
                rstd_s = small.tile([128, NH], F32, tag="rstd_s", name="rstd_s")
                nc.scalar.mul(rstd_s[:, :], rstd[:, :], DESCALE)
                kn = tmp.tile([128, DC], BF16, tag="qr", name="kn")
                for h in range(NH):
                    nc.vector.tensor_scalar_mul(kn[:, h * HD:(h + 1) * HD],
                                                ps[:, h * HD:(h + 1) * HD],
                                                rstd_s[:, h:h + 1])
                for h in range(NH):
                    pt = pstile(f"t{6 + h % 2}", [128, 128], BF16)
                    nc.tensor.transpose(pt[:, :], kn[:, h * HD:(h + 1) * HD],
                                        ident[:, :])
                    nc.vector.tensor_copy(kipT[h][:, :], pt[:, :])
            else:
                for h in range(NH):
                    nc.scalar.mul(vip[:, h, 0:HD],
                                  ps[:, h * HD:(h + 1) * HD], DESCALE)
                    nc.gpsimd.memset(vip[:, h, HD:HD + 1], 1.0)

        # ---- remaining bulk loads ----
        ldqk = {s: wpool.tile([128, NCH, 2 * R], F8, name=f"ldqk{s}")
                for s in (1, 2)}
        ldv = {s: wpool.tile([128, NCH, R], F8, name=f"ldv{s}")
               for s in (1, 2)}
        for s in (1, 2):
            nc.sync.dma_start(out=ldqk[s][:, :, :], in_=din[f"ldqk{s}"])
            nc.sync.dma_start(out=ldv[s][:, :, :], in_=din[f"ldv{s}"])
        luqk = wpool.tile([128, DC], BF16, name="luqk")
        nc.sync.dma_start(out=luqk[0:R, :], in_=din["luq"])
        nc.sync.dma_start(out=luqk[R:2 * R, :], in_=din["luk"])
        luv = wpool.tile([R, DC], BF16, name="luv")
        nc.sync.dma_start(out=luv[:, :], in_=din["luv"])
        lu_rhs = {"q": lambda: luqk[0:R, :], "k": lambda: luqk[R:2 * R, :],
                  "v": lambda: luv[:, :]}

        # ---- q/k/v projections + LoRA + rmsnorm + rope + transposes ----
        # Per-head PE transposes are deferred by one j-tile: the rmsnorm/rope
        # chain takes ~5us after a tile's matmuls stop, and the PE's in-order
        # queue would stall the next tile's matmuls behind not-yet-ready
        # transposes if they were emitted in place.
        ptag = {"q": ("t0", "t1"), "k": ("t2", "t3"), "v": ("t4", "t5")}
        pending_tr = []

        def flush_tr():
            for qr_t, dstT, i_ in pending_tr:
                for h in range(NH):
                    pt = pstile(f"t{6 + h % 2}", [128, 128], BF16)
                    nc.tensor.transpose(pt[:, :],
                                        qr_t[:, h * HD:(h + 1) * HD],
                                        ident[:, :])
                    nc.scalar.copy(dstT[h][:, i_ * 128:(i_ + 1) * 128],
                                   pt[:, :])
            pending_tr.clear()

        for b in range(S // 512):
            s0 = b * 512
            if b == 0:
                hs_t, hs_pair = hs_pre, hs_pre_pair
            else:
                hs_t, hs_pair = load_hs_block(s0)
            if b + 1 < S // 512:
                load_cs(b + 1)
            cond = s0 >= BLOCK
            if cond:
                # rows 0:64 = down_q ranks, 64:128 = down_k ranks
                pd = pstile("t6", [128, 512])
                pdv = pstile("t7", [R, 512])
                for cp in range(NCH // 2):
                    c = 2 * cp
                    last = (c == NCH - 2)
                    for pst, ld in ((pd, ldqk), (pdv, ldv)):
                        nc.tensor.matmul(pst[:, :], lhsT=ld[1][:, c:c + 2, :],
                                         rhs=hs_pair[1](c),
                                         start=(c == 0), stop=False,
                                         perf_mode=DR)
                        nc.tensor.matmul(pst[:, :], lhsT=ld[2][:, c:c + 2, :],
                                         rhs=hs_pair[1](c),
                                         start=False, stop=False, perf_mode=DR)
                        nc.tensor.matmul(pst[:, :], lhsT=ld[1][:, c:c + 2, :],
                                         rhs=hs_pair[2](c),
                                         start=False, stop=last, perf_mode=DR)
                dnqk = small.tile([128, 512], BF16, tag="dnqk", name="dnqk",
                                  bufs=1)
                nc.vector.tensor_copy(dnqk[:, :], pd[:, :])
                dnv = small.tile([R, 512], BF16, tag="dnv", name="dnv", bufs=1)
                nc.vector.tensor_copy(dnv[:, :], pdv[:, :])
                dn_lhsT = {"q": lambda jsl: dnqk[0:R, jsl],
                           "k": lambda jsl: dnqk[R:2 * R, jsl],
                           "v": lambda jsl: dnv[:, jsl]}

            cos_b, sin_b = cs_tiles.pop(b)
            for j in range(4):
                i = b * 4 + j
                js = slice(j * 128, (j + 1) * 128)
                cj = cos_b[:, j]
                sj = sin_b[:, j]
                # stride-0 head-broadcast views [p, NH, ...]
                cos3 = bass.AP(tensor=cj.tensor, offset=cj.offset,
                               ap=[cj.ap[0], [0, NH], [1, HD]])
                sin3p = bass.AP(tensor=sj.tensor, offset=sj.offset,
                                ap=[sj.ap[0], [0, NH], [2, HD // 2], [1, 2]])

                # 3-term DoubleRow accumulation; one stationary hs pair feeds
                # q,k,v before moving to the next chunk pair
                psd = {p: pstile(ptag[p][j % 2], [128, DC]) for p in P}
                for cp in range(NCH // 2):
                    c = 2 * cp
                    last = (c == NCH - 2 and not cond)
                    h1p = hs_pair[1](c)[:, :, js]
                    h2p = hs_pair[2](c)[:, :, js]
                    for p in P:
                        nc.tensor.matmul(psd[p][:, :], lhsT=h1p,
                                         rhs=w_sb[(p, 1)][:, c:c + 2, :],
                                         start=(c == 0), stop=False,
                                         perf_mode=DR)
                        nc.tensor.matmul(psd[p][:, :], lhsT=h1p,
                                         rhs=w_sb[(p, 2)][:, c:c + 2, :],
                                         start=False, stop=False, perf_mode=DR)
                        nc.tensor.matmul(psd[p][:, :], lhsT=h2p,
                                         rhs=w_sb[(p, 1)][:, c:c + 2, :],
                                         start=False, stop=last, perf_mode=DR)
                if cond:
                    for p in P:
                        nc.tensor.matmul(psd[p][:, :], lhsT=dn_lhsT[p](js),
                                         rhs=lu_rhs[p](),
                                         start=False, stop=True)
                flush_tr()
                for p in P:
                    ps = psd[p]
                    if p == "v":
                        # one batched descale+bias into the [128,(3,129)] tile
                        nc.vector.scalar_tensor_tensor(
                            out=v_sb[i][:, :, 0:HD],
                            in0=ps[:, :].rearrange("p (h e) -> p h e", h=NH),
                            scalar=DESCALE,
                            in1=bias["v"][:, :].rearrange("p (h e) -> p h e",
                                                          h=NH),
                            op0=ALU.mult, op1=ALU.add)
                        continue
                    # q/k epilogue: descale+bias (DVE), rmsnorm on ACT
                    # (Square w/ accum_out, then per-head scale-copy), rope
                    # on DVE, per-head transposes (PE) + copies (ACT)
                    qb = tmp.tile([128, DC], F32, tag="qb", name="qb")
                    nc.vector.scalar_tensor_tensor(
                        out=qb[:, :], in0=ps[:, :], scalar=DESCALE,
                        in1=bias[p][:, :], op0=ALU.mult, op1=ALU.add)
                    qn = tmp.tile([128, DC], F32, tag="qn", name="qn")
                    nc.scalar.activation(out=qn[:, :], in_=qb[:, :],
                                         func=AF.Square)
                    msq = small.tile([128, NH], F32, tag="msq", name="msq")
                    nc.vector.tensor_reduce(
                        out=msq[:, :],
                        in_=qn[:, :].rearrange("p (h e) -> p h e", h=NH),
                        axis=mybir.AxisListType.X, op=ALU.add)
                    nc.scalar.activation(out=msq[:, :], in_=msq[:, :],
                                         func=AF.Sqrt,
                                         scale=1.0 / HD, bias=eps_qk[:, :])
                    rstd = small.tile([128, NH], F32, tag="rstd", name="rstd")
                    nc.vector.reciprocal(rstd[:, :], msq[:, :])
                    rs = rstd[:, :]
                    rstd_b = bass.AP(tensor=rs.tensor, offset=rs.offset,
                                     ap=[rs.ap[0], [1, NH], [0, HD]])
                    nc.vector.tensor_tensor(
                        out=qn[:, :].rearrange("p (h e) -> p h e", h=NH),
                        in0=qb[:, :].rearrange("p (h e) -> p h e", h=NH),
                        in1=rstd_b, op=ALU.mult)
                    # rope: m1 = qn*cos ; m2 = revpairs(qn)*sin_signfolded
                    m1 = tmp.tile([128, DC], F32, tag="sq", name="m1")
                    nc.vector.tensor_tensor(
                        out=m1[:, :].rearrange("p (h e) -> p h e", h=NH),
                        in0=qn[:, :].rearrange("p (h e) -> p h e", h=NH),
                        in1=cos3, op=ALU.mult)
                    m2 = tmp.tile([128, DC], F32, tag="qb", name="m2")
                    qnr = qn[:, :]
                    qn_rev = bass.AP(tensor=qnr.tensor, offset=qnr.offset + 1,
                                     ap=[qnr.ap[0], [HD, NH], [2, HD // 2],
                                         [-1, 2]])
                    nc.vector.tensor_tensor(
                        out=m2[:, :].rearrange("p (h i two) -> p h i two",
                                               h=NH, two=2),
                        in0=qn_rev, in1=sin3p, op=ALU.mult)
                    qr = tmp.tile([128, DC], BF16, tag="qr", name="qr")
                    nc.vector.tensor_tensor(out=qr[:, :], in0=m1[:, :],
                                            in1=m2[:, :], op=ALU.add)
                    pending_tr.append((qr, qT if p == "q" else kT, i))

        flush_tr()

        # ===== phase 2: attention =====
        # close phase-1 PSUM pool, open phase-2 layout:
        #   sc0/sc1: [128,1024] score pairs (2 banks each)
        #   c0..c3:  [128,258] attn+ip accumulators (1 bank each)
        ps1_ctx.__exit__(None, None, None)
        ps2_ctx = tc.tile_pool(name="ps2", bufs=1, space="PSUM")
        ps2 = ps2_ctx.__enter__()
        scn = 0
        for h in range(NH):
            for sbk in range(S // 512):
                s0 = sbk * 512
                tcs = (list(range(NCH)) if s0 < BLOCK
                       else list(range(BLOCK // 128, NCH)))
                pairs = [(tcs[x], tcs[x + 1]) for x in range(0, len(tcs), 2)]

                # main pairs, software-pipelined with 1 pair of score lookahead
                def sc_exp(pr):
                    nonlocal scn
                    tka, tkb = pr
                    psc = ps2.tile([128, 1024], F32, tag=f"sc{scn % 2}",
                                   name=f"sc{scn % 2}")
                    scn += 1
                    for half, tk in ((0, tka), (1, tkb)):
                        nc.tensor.matmul(psc[:, half * 512:(half + 1) * 512],
                                         lhsT=kT[h][:, tk * 128:(tk + 1) * 128],
                                         rhs=qT[h][:, s0:s0 + 512],
                                         start=True, stop=True,
                                         skip_group_check=True)
                    pT = pt2.tile([128, 1024], BF16, tag="pT", name="pT")
                    nc.scalar.activation(out=pT[:, :], in_=psc[:, :],
                                         func=AF.Exp, scale=SM)
                    return pT

                pT_q = [sc_exp(pairs[0])]
                cmb = [ps2.tile([128, 2 * (HD + 1)], F32, tag=f"c{j}",
                                name=f"c{j}") for j in range(4)]

                # ip branch: emitted after pair-0's score/exp (so ACT starts the
                # main exp stream immediately) but its attnV still precedes main
                # pair-0's start=True — ip writes cols 129:258 once; the later
                # main start clears only has_written bits, not data.
                psc = ps2.tile([128, 1024], F32, tag=f"sc{scn % 2}",
                               name=f"sc{scn % 2}")
                scn += 1
                nc.tensor.matmul(psc[:, 0:512], lhsT=kipT[h][:, :],
                                 rhs=qT[h][:, s0:s0 + 512],
                                 start=True, stop=True, skip_group_check=True)
                pTip = pt2.tile([128, 1024], BF16, tag="pT", name="pT")
                nc.scalar.activation(out=pTip[:, 0:512], in_=psc[:, 0:512],
                                     func=AF.Exp, scale=SM)
                for j in range(4):
                    nc.tensor.matmul(cmb[j][:, HD + 1:2 * (HD + 1)],
                                     lhsT=pTip[:, j * 128:(j + 1) * 128],
                                     rhs=vip[:, h, :],
                                     start=True, stop=True,
                                     skip_group_check=True)

                for pi2 in range(len(pairs)):
                    if pi2 + 1 < len(pairs):
                        pT_q.append(sc_exp(pairs[pi2 + 1]))
                    pT = pT_q.pop(0)
                    tka, tkb = pairs[pi2]
                    for half, tk in ((0, tka), (1, tkb)):
                        for j in range(4):
                            nc.tensor.matmul(
                                cmb[j][:, 0:HD + 1],
                                lhsT=pT[:, half * 512 + j * 128:
                                        half * 512 + (j + 1) * 128],
                                rhs=v_sb[tk][:, h, :],
                                start=(pi2 == 0 and half == 0),
                                stop=(pi2 == len(pairs) - 1 and half == 1),
                                skip_group_check=True)
                # normalize + combine + one batched store
                o2 = op2.tile([128, 4, HD], BF16, tag="o2", name="o2", bufs=2)
                for j in range(4):
                    rm = osm.tile([128, 1], F32, tag="rm", name="rm")
                    nc.vector.reciprocal(rm[:, :], cmb[j][:, HD:HD + 1])
                    ri = osm.tile([128, 1], F32, tag="ri", name="ri")
                    nc.vector.reciprocal(ri[:, :],
                                         cmb[j][:, 2 * HD + 1:2 * HD + 2])
                    nc.vector.tensor_scalar_mul(o2[:, j, :], cmb[j][:, 0:HD],
                                                rm[:, :])
                    nc.vector.scalar_tensor_tensor(
                        out=o2[:, j, :], in0=cmb[j][:, HD + 1:2 * HD + 1],
                        scalar=ri[:, :], in1=o2[:, j, :],
                        op0=ALU.mult, op1=ALU.add)
                nc.sync.dma_start(
                    out=bass.AP(tensor=out_d.tensor,
                                offset=s0 * DC + h * HD,
                                ap=[[DC, 128], [128 * DC, 4], [1, HD]]),
                    in_=o2[:, :, :])
        ps2_ctx.__exit__(None, None, None)


def _split8(x, scale):
    """fp8e4 residual split of `x*scale`: returns (x1, x2) with
    x1 + x2 ~= x*scale to ~2^-9 relative."""
    xs = np.ascontiguousarray(x, dtype=np.float32) * np.float32(scale)
    a = xs.astype(ml_dtypes.float8_e4m3)
    b = (xs - a.astype(np.float32)).astype(ml_dtypes.float8_e4m3)
    return a, b


def _pm(x):
    """[D, C] -> partition-major [128, (D//128)*C] so each SBUF partition's
    data is one contiguous DMA run."""
    d, c = x.shape
    return np.ascontiguousarray(
        x.reshape(d // 128, 128, c).transpose(1, 0, 2).reshape(128, -1))


def _prep_inputs(inputs):
    # Inputs may be jax arrays: np.asarray without an explicit dtype hits the
    # jax Array's cached host copy (an explicit dtype forces a fresh transfer).
    inp = {k: np.asarray(v) for k, v in inputs.items()}
    bf = lambda x: np.ascontiguousarray(x).astype(ml_dtypes.bfloat16)
    f32 = lambda x: np.ascontiguousarray(x, dtype=np.float32)
    hs1, hs2 = _split8(inp["hidden_states"][0].T, HS_S)
    ie1, ie2 = _split8(_pm(f32(inp["image_emb"][0].T)), HS_S)
    cos = bf(_pm(f32(inp["rope_cos"])))
    sin_sf = np.array(inp["rope_sin"], dtype=np.float32)
    sin_sf[:, 0::2] *= -1.0
    sinf = bf(_pm(sin_sf))
    # pre-transpose the full matrices once, then slice + partition-major tile
    wT = {k: f32(inp[k].T) for k in ("wq", "wk", "wv", "w_kip", "w_vip")}
    luT = {k: bf(inp[k].T) for k in ("lq_up", "lk_up", "lv_up")}
    # lora-down: [R, D] -> [D, R] -> interleaved [128, NCH*(2R|R)] tiles
    ldq_t = f32(inp["lq_down"].T).reshape(NCH, 128, R)
    ldk_t = f32(inp["lk_down"].T).reshape(NCH, 128, R)
    ldqk = np.concatenate([ldq_t, ldk_t], axis=2).transpose(1, 0, 2)
    ldqk1, ldqk2 = _split8(np.ascontiguousarray(ldqk.reshape(128, -1)), W_S)
    ldv1, ldv2 = _split8(_pm(f32(inp["lv_down"].T)), W_S)
    in_maps = []
    for c in range(N_CORES):
        sl = slice(DC * c, DC * (c + 1))
        m = {
            "hsT1": hs1, "hsT2": hs2, "iembT1": ie1, "iembT2": ie2,
            "cos": cos, "sinf": sinf,
            "bq": f32(inp["bq"][sl]),
            "bk": f32(inp["bk"][sl]),
            "bv": f32(inp["bv"][sl]),
            "luq": np.ascontiguousarray(luT["lq_up"][:, sl]),
            "luk": np.ascontiguousarray(luT["lk_up"][:, sl]),
            "luv": np.ascontiguousarray(luT["lv_up"][:, sl]),
            "ldqk1": ldqk1, "ldqk2": ldqk2, "ldv1": ldv1, "ldv2": ldv2,
        }
        for base, key in [("wqT", "wq"), ("wkT", "wk"), ("wvT", "wv"),
                          ("wkipT", "w_kip"), ("wvipT", "w_vip")]:
            w1, w2 = _split8(_pm(wT[key][:, sl]), W_S)
            m[base + "1"], m[base + "2"] = w1, w2
        in_maps.append(m)
    return in_maps


def kernel(**inputs):
    if "nc" not in _BUILT:
        _BUILT["nc"] = _build()
    nc = _BUILT["nc"]
    in_maps = _prep_inputs(inputs)
    res = run_bass_kernel_spmd(nc, in_maps, list(range(N_CORES))).results
    out = np.concatenate([res[c]["out"].astype(np.float32)
                          for c in range(N_CORES)], axis=-1)
    return out.reshape(1, S, D)
